# revision 1
# baseline (speedup 1.0000x reference)
"""Trainium2 Bass kernel for nn_MultiHeadDynamics.

Computation (per sample row x of state, s of signal):
    heads   = x.reshape(H, DH)                      # H=16, DH=256
    A_h     = U_h @ V_h + diag(d_h)                 # (DH, DH) per head
    lin     = heads @ A_h^T
    c       = heads - mean_dh(heads)
    drift   = lin + cs * c^3 + s
    out     = x + DT*(1+cp)*drift - (DT*cp/H) * sum_h(drift_h)

Folding:  beta = DT*(1+cp);  gp = DT*cp/(H*beta);  sq = sqrt(beta*cs)
    D'      = beta*drift = heads @ (beta*A)^T + Square(sq*c)*c + beta*s
    out     = x + D' - gp * sum_h(D'_h)

Sharding: batch B=8192 split across 8 cores (1024 rows each), params
replicated. Per core, rows are processed as 8 tiles of [128, 4096].
The head contraction needs d on partitions, so each [128,128] chunk of
the state tile is PE-transposed; transposed chunks serve as matmul
stationary operands against precomputed (beta*A)^T, with a fused
ones-vector matmul producing the within-head means for free.
"""

import sys

for _p in ("/opt/trn_rl_repo",):
    if _p not in sys.path:
        sys.path.insert(0, _p)

import math
from contextlib import ExitStack

import numpy as np

import concourse.bass as bass
import concourse.tile as tile
from concourse import bacc, mybir
from concourse.bass_utils import run_bass_kernel_spmd
from concourse.masks import make_identity

F32 = mybir.dt.float32
AOP = mybir.AluOpType

# Problem constants (full-input shapes; hardcoded per the task contract).
B = 8192
D = 4096
H = 16
DH = 256
R = 64
DT = 0.05
NCORES = 8
BS = B // NCORES          # rows per core = 1024
P = 128                   # partitions
NT = BS // P              # row tiles per core = 8
NCH = D // P              # 128-wide column chunks per row tile = 32

# Matmul dtype: bfloat16 keeps PE fast (1 cyc/row) with ~4e-5 output
# error; float32 is exact but 4 cyc/row.
MM_DTYPE = mybir.dt.bfloat16
BF16 = mybir.dt.bfloat16
# Middle elementwise chain dtype: fp16 has a 10-bit mantissa (8x finer
# than bf16) and still gets the DVE 16-bit 2x packing mode.
MID = mybir.dt.float16

# Columns of the final fp32 (x + dd) pass handled by DVE; the rest on
# GpSimd. fp32 tensor_tensor is 1x on DVE, ~2x worse on GpSimd.
FINAL_DVE_COLS = 1024

# Fold drift = lin + t2 into PSUM via identity matmuls on the PE
# (software-pipelined one tile behind so the PE never waits on t2).
IDENT_MM = True


def _emit(tc: tile.TileContext, aps: dict, cubic_scale: float, coupling: float):
    nc = tc.nc
    beta = DT * (1.0 + coupling)
    gp = DT * coupling / (H * beta)
    sq = math.sqrt(beta * cubic_scale)

    state = aps["state"]
    signal = aps["signal"]
    U_d = aps["U"]
    V_d = aps["V"]
    diag_d = aps["diag"]
    out_d = aps["out"]

    with ExitStack() as ctx:
        consts = ctx.enter_context(tc.tile_pool(name="consts", bufs=1))

        ident = consts.tile([P, P], F32, tag="ident")
        make_identity(nc, ident)
        ident_bf = consts.tile([P, P], BF16, tag="ident_bf")
        make_identity(nc, ident_bf)


        # Diagonal-position masks for the two 128-chunks of a head.
        dmasks = []
        for k in range(2):
            dmask = consts.tile([P, DH], F32, tag=f"dmask{k}")
            nc.gpsimd.memset(dmask, 0.0)
            nc.gpsimd.affine_select(
                out=dmask, in_=dmask,
                compare_op=AOP.not_equal, fill=1.0,
                base=-(k * P), pattern=[[1, DH]], channel_multiplier=-1,
            )
            dmasks.append(dmask)

        ones = consts.tile([P, 1], MM_DTYPE, tag="ones")
        nc.gpsimd.memset(ones, 1.0 / DH)

        # (beta*A)^T, laid out [d-chunk partition, head, chunk, e].
        AT = consts.tile([P, H, 2, DH], MM_DTYPE, tag="AT")

        # --- one-time A setup ---
        with (
            tc.tile_pool(name="setup", bufs=2) as setup,
            tc.tile_pool(name="setup_ps", bufs=2, space="PSUM") as setup_ps,
        ):
            for h in range(H):
                u_s = setup.tile([P, 2, R], F32, tag="u_s")
                nc.sync.dma_start(out=u_s, in_=U_d[h].rearrange("(k p) r -> p k r", p=P))
                v_s = setup.tile([R, DH], F32, tag="v_s")
                nc.sync.dma_start(out=v_s, in_=V_d[h])
                dcol = setup.tile([P, 2], F32, tag="dcol")
                nc.sync.dma_start(
                    out=dcol, in_=diag_d[h].rearrange("(k p) -> p k", p=P)
                )

                # U_h^T via PE transpose: [128,64] chunks -> [64,128]
                ut_s = setup.tile([R, DH], F32, tag="ut_s")
                for k in range(2):
                    ut_ps = setup_ps.tile([R, P], F32, tag="ut_ps")
                    nc.tensor.transpose(ut_ps, u_s[:, k, :], ident)
                    nc.scalar.copy(out=ut_s[:, k * P:(k + 1) * P], in_=ut_ps)

                for k in range(2):
                    # (V^T U^T) chunk: [d=128, e=256]
                    a_ps = setup_ps.tile([P, DH], F32, tag="a_ps")
                    nc.tensor.matmul(
                        a_ps, lhsT=v_s[:, k * P:(k + 1) * P], rhs=ut_s,
                        start=True, stop=True,
                    )
                    # beta * diag embedded on the diagonal of this chunk
                    dg = setup.tile([P, DH], F32, tag="dg")
                    nc.vector.tensor_scalar(
                        out=dg, in0=dmasks[k],
                        scalar1=dcol[:, k:k + 1], scalar2=beta,
                        op0=AOP.mult, op1=AOP.mult,
                    )
                    # AT[:, h, k, :] = beta*(V^T U^T) + beta*diag, cast
                    nc.vector.scalar_tensor_tensor(
                        out=AT[:, h, k, :], in0=a_ps, scalar=beta, in1=dg,
                        op0=AOP.mult, op1=AOP.add,
                    )

        # --- main loop pools ---
        xp = ctx.enter_context(tc.tile_pool(name="xp", bufs=3))
        sp = ctx.enter_context(tc.tile_pool(name="sp", bufs=2))
        tp = ctx.enter_context(tc.tile_pool(name="tp", bufs=1))
        hp = ctx.enter_context(tc.tile_pool(name="hp", bufs=2))
        mp = ctx.enter_context(tc.tile_pool(name="mp", bufs=2))
        trp = ctx.enter_context(tc.tile_pool(name="trp", bufs=2))
        ps_tp = ctx.enter_context(tc.tile_pool(name="ps_tp", bufs=2, space="PSUM"))
        ps_lin = ctx.enter_context(tc.tile_pool(name="ps_lin", bufs=3, space="PSUM"))
        ps_m = ctx.enter_context(tc.tile_pool(name="ps_m", bufs=1, space="PSUM"))

        for it in range(NT):
            r0 = it * P
            # split input streams across the two HWDGE queues (SP / ACT)
            x_t = xp.tile([P, D], F32, tag="x", name="x_t")
            nc.sync.dma_start(out=x_t, in_=state[r0:r0 + P, :])
            s_t = sp.tile([P, D], F32, tag="s", name="s_t")
            nc.scalar.dma_start(out=s_t, in_=signal[r0:r0 + P, :])

            # beta*s in fp16, off the critical chain (DVE 2x)
            sb_t = tp.tile([P, D], MID, tag="sb", name="sb_t")
            nc.vector.tensor_scalar(
                out=sb_t, in0=s_t, scalar1=beta, scalar2=None, op0=AOP.mult,
            )

            # Transpose all 32 f32 chunks of x into hT (d on partitions),
            # casting to bf16 in the PSUM->SBUF copy.
            hT = hp.tile([P, NCH, P], MM_DTYPE, tag="hT", name="hT")
            for g in range(NCH // 8):
                tp_ps = ps_tp.tile([P, 8 * P], F32, tag="tp_ps", name="tp_ps")
                for c8 in range(8):
                    j = g * 8 + c8
                    nc.tensor.transpose(
                        tp_ps[:, c8 * P:(c8 + 1) * P],
                        x_t[:, j * P:(j + 1) * P], ident,
                    )
                nc.scalar.copy(
                    out=hT[:, g * 8:(g + 1) * 8, :].rearrange("p a b -> p (a b)"),
                    in_=tp_ps,
                )

            # Per-head matmuls: lin' per head pair; within-head means via
            # the ones vector (value 1/DH) as an extra cheap matmul.
            m_ps = ps_m.tile([P, H], F32, tag="m_ps", name="m_ps")
            lin_t = tp.tile([P, D], MID, tag="lin", name="lin_t")
            for hp2 in range(H // 2):
                l_ps = ps_lin.tile([P, 2 * DH], F32, tag="l_ps", name="l_ps")
                for hh in range(2):
                    h = hp2 * 2 + hh
                    for k in range(2):
                        j = 2 * h + k
                        nc.tensor.matmul(
                            l_ps[:, hh * DH:(hh + 1) * DH],
                            lhsT=hT[:, j, :], rhs=AT[:, h, k, :],
                            start=(k == 0), stop=(k == 1),
                        )
                        nc.tensor.matmul(
                            m_ps[:, h:h + 1],
                            lhsT=hT[:, j, :], rhs=ones,
                            start=(k == 0), stop=(k == 1),
                        )
                nc.scalar.copy(
                    out=lin_t[:, hp2 * 2 * DH:(hp2 + 1) * 2 * DH], in_=l_ps
                )
            m_t = mp.tile([P, H], F32, tag="m", name="m_t")
            nc.scalar.copy(out=m_t, in_=m_ps)
            msq_t = mp.tile([P, H], F32, tag="msq", name="msq_t")
            nc.scalar.mul(msq_t, m_ps, -sq)

            # c2 = beta*cs*(x-m)^2 straight from x on ACT (bias trick)
            c2_t = tp.tile([P, D], MID, tag="c2", name="c2_t")
            for h in range(H):
                nc.scalar.activation(
                    out=c2_t[:, h * DH:(h + 1) * DH],
                    in_=x_t[:, h * DH:(h + 1) * DH],
                    func=mybir.ActivationFunctionType.Square,
                    scale=sq, bias=msq_t[:, h:h + 1],
                )
            # c3 = (x - m) * c2 fused per segment
            c3_t = tp.tile([P, D], MID, tag="c3", name="c3_t")
            for h in range(H):
                nc.vector.scalar_tensor_tensor(
                    out=c3_t[:, h * DH:(h + 1) * DH],
                    in0=x_t[:, h * DH:(h + 1) * DH],
                    scalar=m_t[:, h:h + 1],
                    in1=c2_t[:, h * DH:(h + 1) * DH],
                    op0=AOP.subtract, op1=AOP.mult,
                )
            # t2 = beta*s + c3 (fp16 2x)
            t2_t = tp.tile([P, D], MID, tag="t2", name="t2_t")
            nc.vector.tensor_add(t2_t, sb_t, c3_t)
            # drift = lin' + t2 (fp16 2x; reuse c3's buffer)
            dr_t = c3_t
            nc.vector.tensor_add(dr_t, lin_t, t2_t)

            # head-sum tree, flat contiguous halves (order-independent sum)
            t8 = trp.tile([P, D // 2], MID, tag="t8", name="t8")
            nc.vector.tensor_add(t8, dr_t[:, 0:D // 2], dr_t[:, D // 2:D])
            t4 = trp.tile([P, D // 4], MID, tag="t4", name="t4")
            nc.vector.tensor_add(t4, t8[:, 0:D // 4], t8[:, D // 4:D // 2])
            t2r = trp.tile([P, D // 8], MID, tag="t2r", name="t2r")
            nc.vector.tensor_add(t2r, t4[:, 0:D // 8], t4[:, D // 8:D // 4])
            # mhn2 = two side-by-side copies of -gp*sum_h(drift)
            mhn2 = trp.tile([P, 2 * DH], MID, tag="mhn2", name="mhn2")
            nc.vector.tensor_add(mhn2[:, 0:DH], t2r[:, 0:DH], t2r[:, DH:2 * DH])
            nc.vector.tensor_scalar_mul(mhn2[:, 0:DH], mhn2[:, 0:DH], -gp)
            nc.vector.tensor_copy(mhn2[:, DH:2 * DH], mhn2[:, 0:DH])

            # dd = drift + mhn (head-pair flat adds; reuse c2's buffer)
            dd_t = c2_t
            for hp2 in range(H // 2):
                nc.vector.tensor_add(
                    dd_t[:, hp2 * 2 * DH:(hp2 + 1) * 2 * DH],
                    dr_t[:, hp2 * 2 * DH:(hp2 + 1) * 2 * DH], mhn2,
                )

            # out = x + dd (fp32+fp16 mixed, split DVE / GpSimd; into x buf)
            o_t = x_t
            ncol = FINAL_DVE_COLS
            if ncol > 0:
                nc.vector.tensor_add(
                    o_t[:, 0:ncol], x_t[:, 0:ncol], dd_t[:, 0:ncol]
                )
            if ncol < D:
                nc.gpsimd.tensor_add(
                    o_t[:, ncol:D], x_t[:, ncol:D], dd_t[:, ncol:D]
                )
            if it % 2 == 0:
                nc.sync.dma_start(out=out_d[r0:r0 + P, :], in_=o_t)
            else:
                nc.scalar.dma_start(out=out_d[r0:r0 + P, :], in_=o_t)



_CACHE: dict = {}


def _build(cubic_scale: float, coupling: float) -> bass.Bass:
    key = (float(cubic_scale), float(coupling), MM_DTYPE, FINAL_DVE_COLS)
    if key in _CACHE:
        return _CACHE[key]
    nc = bacc.Bacc("TRN2", target_bir_lowering=False, debug=False)
    aps = {
        "state": nc.dram_tensor("state", [BS, D], F32, kind="ExternalInput").ap(),
        "signal": nc.dram_tensor("signal", [BS, D], F32, kind="ExternalInput").ap(),
        "U": nc.dram_tensor("U", [H, DH, R], F32, kind="ExternalInput").ap(),
        "V": nc.dram_tensor("V", [H, R, DH], F32, kind="ExternalInput").ap(),
        "diag": nc.dram_tensor("diag", [H, DH], F32, kind="ExternalInput").ap(),
        "out": nc.dram_tensor("out", [BS, D], F32, kind="ExternalOutput").ap(),
    }
    with tile.TileContext(nc) as tc:
        _emit(tc, aps, float(cubic_scale), float(coupling))
    nc.compile()
    _CACHE[key] = nc
    return nc


def run(state, signal, U, V, diag, cubic_scale, coupling, trace=False):
    state = np.ascontiguousarray(np.asarray(state, dtype=np.float32))
    signal = np.ascontiguousarray(np.asarray(signal, dtype=np.float32))
    U = np.ascontiguousarray(np.asarray(U, dtype=np.float32))
    V = np.ascontiguousarray(np.asarray(V, dtype=np.float32))
    diag = np.ascontiguousarray(np.asarray(diag, dtype=np.float32))

    nc = _build(float(cubic_scale), float(coupling))
    in_maps = []
    for i in range(NCORES):
        sl = slice(i * BS, (i + 1) * BS)
        in_maps.append({
            "state": state[sl], "signal": signal[sl],
            "U": U, "V": V, "diag": diag,
        })
    res = run_bass_kernel_spmd(nc, in_maps, list(range(NCORES)), trace=trace)
    out = np.concatenate([res.results[i]["out"] for i in range(NCORES)], axis=0)
    return out, res


def kernel(state, signal, U, V, diag, cubic_scale, coupling) -> np.ndarray:
    out, _ = run(state, signal, U, V, diag, cubic_scale, coupling, trace=False)
    return out



# revision 6
# speedup vs baseline: 1.3616x; 1.3616x over previous
"""Trainium2 Bass kernel for nn_MultiHeadDynamics (v2, fp16 IO).

Computation (per sample row x of state, s of signal):
    heads   = x.reshape(H, DH)                      # H=16, DH=256
    A_h     = U_h @ V_h + diag(d_h)                 # (DH, DH) per head
    lin     = heads @ A_h^T
    c       = heads - mean_dh(heads)
    drift   = lin + cs * c^3 + s
    out     = x + DT*(1+cp)*drift - (DT*cp/H) * sum_h(drift_h)

Folding:  beta = DT*(1+cp);  gp = DT*cp/(H*beta);  gam = cbrt(beta*cs)
    D'      = beta*drift = heads @ (beta*A)^T + (gam*c)^3 + beta*s
    out     = x + D' - gp * sum_h(D'_h)

v2 design:
  - IO in fp16: the host casts state/signal to fp16 and upcasts the
    fp16 result; halves HBM traffic (tolerance gate is 2e-2, fp16
    pipeline measures ~7e-4).
  - PE: per-chunk fp16 transposes; per-head matmuls against (beta*A)^T
    with free N=1 mean-extractor matmuls; identity-stationary matmuls
    accumulate beta*s and c3 into the same PSUM region so drift is
    completed inside PSUM (saves two full-tile DVE passes).
  - ACT: full-tile Square for c2 (ct is already centered, no bias) and
    PSUM->SBUF drift copies.
  - DVE: per-head ct = gam*(x-m), c3 = ct*c2, head-coupling tree,
    dd broadcast add, final out = x + dd.
  - GpSimd: transposed-chunk PSUM->SBUF copies + small ops.

Sharding: batch B=8192 split across 8 cores (1024 rows each), params
replicated; per core 8 row tiles of [128, 4096].
"""

import sys

for _p in ("/opt/trn_rl_repo",):
    if _p not in sys.path:
        sys.path.insert(0, _p)

import math
from contextlib import ExitStack

import numpy as np

import concourse.bass as bass
import concourse.tile as tile
from concourse import bacc, mybir
from concourse.bass_utils import run_bass_kernel_spmd
from concourse.masks import make_identity

F32 = mybir.dt.float32
FP16 = mybir.dt.float16
AOP = mybir.AluOpType

B = 8192
D = 4096
H = 16
DH = 256
R = 64
DT = 0.05
NCORES = 8
BS = B // NCORES          # rows per core = 1024
P = 128                   # partitions
NT = BS // P              # row tiles per core = 8
NCH = D // P              # 128-wide column chunks per row tile = 32

# --- tuning flags ---
FOLD_S = True             # accumulate beta*s into lin PSUM on the PE
FOLD_C3 = True            # accumulate c3 into lin PSUM on the PE
# engine per transposed-chunk-group PSUM->SBUF copy (4 groups of 8)
# (GpSimd cannot access PSUM, so only scalar/vector are valid here)
HT_COPY_ENG = ("scalar", "vector", "scalar", "vector")
# engine per pair drift PSUM->SBUF copy (8 pairs)
DR_COPY_ENG = ("scalar",) * 8


def _copy(nc, eng, out, in_):
    if eng == "scalar":
        nc.scalar.copy(out=out, in_=in_)
    elif eng == "vector":
        nc.vector.tensor_copy(out, in_)
    else:
        nc.gpsimd.tensor_copy(out, in_)


def _emit(tc: tile.TileContext, aps: dict, cubic_scale: float, coupling: float):
    nc = tc.nc
    beta = DT * (1.0 + coupling)
    gp = DT * coupling / (H * beta)
    gam = (beta * cubic_scale) ** (1.0 / 3.0)

    state = aps["state"]
    signal = aps["signal"]
    U_d = aps["U"]
    V_d = aps["V"]
    diag_d = aps["diag"]
    out_d = aps["out"]

    with ExitStack() as ctx:
        consts = ctx.enter_context(tc.tile_pool(name="consts", bufs=1))

        ident = consts.tile([P, P], F32, tag="ident")
        make_identity(nc, ident)
        ident16 = consts.tile([P, P], FP16, tag="ident16")
        make_identity(nc, ident16)
        identb = consts.tile([P, P], FP16, tag="identb")
        make_identity(nc, identb)
        nc.vector.tensor_scalar(
            out=identb, in0=identb, scalar1=beta, scalar2=None, op0=AOP.mult
        )

        # Diagonal-position masks for the two 128-chunks of a head.
        dmasks = []
        for k in range(2):
            dmask = consts.tile([P, DH], F32, tag=f"dmask{k}")
            nc.gpsimd.memset(dmask, 0.0)
            nc.gpsimd.affine_select(
                out=dmask, in_=dmask,
                compare_op=AOP.not_equal, fill=1.0,
                base=-(k * P), pattern=[[1, DH]], channel_multiplier=-1,
            )
            dmasks.append(dmask)

        ones = consts.tile([P, 1], FP16, tag="ones")
        nc.gpsimd.memset(ones, 1.0 / DH)

        # (beta*A)^T, laid out [d-chunk partition, head, chunk, e].
        AT = consts.tile([P, H, 2, DH], FP16, tag="AT")

        # --- one-time A setup (f32 math, cast to fp16 at the end) ---
        with (
            tc.tile_pool(name="setup", bufs=2) as setup,
            tc.tile_pool(name="setup_ps", bufs=2, space="PSUM") as setup_ps,
        ):
            for h in range(H):
                u_s = setup.tile([P, 2, R], F32, tag="u_s")
                nc.sync.dma_start(out=u_s, in_=U_d[h].rearrange("(k p) r -> p k r", p=P))
                v_s = setup.tile([R, DH], F32, tag="v_s")
                nc.sync.dma_start(out=v_s, in_=V_d[h])
                dcol = setup.tile([P, 2], F32, tag="dcol")
                nc.sync.dma_start(
                    out=dcol, in_=diag_d[h].rearrange("(k p) -> p k", p=P)
                )

                # U_h^T via PE transpose: [128,64] chunks -> [64,128]
                ut_s = setup.tile([R, DH], F32, tag="ut_s")
                for k in range(2):
                    ut_ps = setup_ps.tile([R, P], F32, tag="ut_ps")
                    nc.tensor.transpose(ut_ps, u_s[:, k, :], ident)
                    nc.scalar.copy(out=ut_s[:, k * P:(k + 1) * P], in_=ut_ps)

                for k in range(2):
                    # (V^T U^T) chunk: [d=128, e=256]
                    a_ps = setup_ps.tile([P, DH], F32, tag="a_ps")
                    nc.tensor.matmul(
                        a_ps, lhsT=v_s[:, k * P:(k + 1) * P], rhs=ut_s,
                        start=True, stop=True,
                    )
                    # beta * diag embedded on the diagonal of this chunk
                    dg = setup.tile([P, DH], F32, tag="dg")
                    nc.vector.tensor_scalar(
                        out=dg, in0=dmasks[k],
                        scalar1=dcol[:, k:k + 1], scalar2=beta,
                        op0=AOP.mult, op1=AOP.mult,
                    )
                    # AT[:, h, k, :] = beta*(V^T U^T) + beta*diag, cast fp16
                    nc.vector.scalar_tensor_tensor(
                        out=AT[:, h, k, :], in0=a_ps, scalar=beta, in1=dg,
                        op0=AOP.mult, op1=AOP.add,
                    )

        # --- main loop pools ---
        xp = ctx.enter_context(tc.tile_pool(name="xp", bufs=3))
        sp = ctx.enter_context(tc.tile_pool(name="sp", bufs=2))
        hp = ctx.enter_context(tc.tile_pool(name="hp", bufs=2))
        cp_ = ctx.enter_context(tc.tile_pool(name="cp", bufs=2))
        dp = ctx.enter_context(tc.tile_pool(name="dp", bufs=2))
        mp = ctx.enter_context(tc.tile_pool(name="mp", bufs=2))
        trp = ctx.enter_context(tc.tile_pool(name="trp", bufs=2))
        op_ = ctx.enter_context(tc.tile_pool(name="op", bufs=2))
        ps_tp = ctx.enter_context(tc.tile_pool(name="ps_tp", bufs=2, space="PSUM"))
        ps_lin = ctx.enter_context(tc.tile_pool(name="ps_lin", bufs=4, space="PSUM"))
        ps_m = ctx.enter_context(tc.tile_pool(name="ps_m", bufs=2, space="PSUM"))

        for it in range(NT):
            r0 = it * P
            x_t = xp.tile([P, D], FP16, tag="x", name="x_t")
            nc.sync.dma_start(out=x_t, in_=state[r0:r0 + P, :])
            s_t = sp.tile([P, D], FP16, tag="s", name="s_t")
            nc.scalar.dma_start(out=s_t, in_=signal[r0:r0 + P, :])

            # Transpose all 32 fp16 chunks of x into hT (d on partitions).
            hT = hp.tile([P, NCH, P], FP16, tag="hT", name="hT")
            for g in range(NCH // 8):
                tp_ps = ps_tp.tile([P, 8 * P], FP16, tag="tp_ps", name="tp_ps")
                for c8 in range(8):
                    j = g * 8 + c8
                    nc.tensor.transpose(
                        tp_ps[:, c8 * P:(c8 + 1) * P],
                        x_t[:, j * P:(j + 1) * P], ident16,
                    )
                _copy(
                    nc, HT_COPY_ENG[g],
                    hT[:, g * 8:(g + 1) * 8, :].rearrange("p a b -> p (a b)"),
                    tp_ps,
                )

            # Per-head matmuls + free mean extractors; PE accumulates
            # beta*s (and later c3) into the same PSUM region.
            m_ps = ps_m.tile([P, H], F32, tag="m_ps", name="m_ps")
            m_t = mp.tile([P, H], F32, tag="m", name="m_t")
            ct_t = cp_.tile([P, D], FP16, tag="ct", name="ct_t")
            c2_t = cp_.tile([P, D], FP16, tag="c2", name="c2_t")
            c3_t = cp_.tile([P, D], FP16, tag="c3", name="c3_t")
            dr_t = dp.tile([P, D], FP16, tag="dr", name="dr_t")
            # Two half-tile phases of 4 head-pairs each: ps_lin has 4 bufs,
            # so a half's 4 PSUM tiles stay live until its c3-folds run.
            for half in range(2):
                l_pss = []
                for hp2 in range(4 * half, 4 * half + 4):
                    l_ps = ps_lin.tile([P, 2 * DH], F32, tag="l_ps", name="l_ps")
                    l_pss.append(l_ps)
                    for hh in range(2):
                        h = hp2 * 2 + hh
                        for k in range(2):
                            j = 2 * h + k
                            nc.tensor.matmul(
                                l_ps[:, hh * DH:(hh + 1) * DH],
                                lhsT=hT[:, j, :], rhs=AT[:, h, k, :],
                                start=(k == 0),
                                stop=(k == 1) and not (FOLD_S or FOLD_C3),
                            )
                            nc.tensor.matmul(
                                m_ps[:, h:h + 1],
                                lhsT=hT[:, j, :], rhs=ones,
                                start=(k == 0), stop=(k == 1),
                            )
                    if FOLD_S:
                        for hh in range(2):
                            h = hp2 * 2 + hh
                            nc.tensor.matmul(
                                l_ps[:, hh * DH:(hh + 1) * DH],
                                lhsT=identb, rhs=s_t[:, h * DH:(h + 1) * DH],
                                start=False,
                                stop=(hh == 1) and not FOLD_C3,
                                skip_group_check=True,
                            )

                    # mean for this pair to SBUF (fp16), then ct per head
                    nc.vector.tensor_copy(
                        m_t[:, 2 * hp2:2 * hp2 + 2], m_ps[:, 2 * hp2:2 * hp2 + 2]
                    )
                    for hh in range(2):
                        h = hp2 * 2 + hh
                        nc.vector.tensor_scalar(
                            out=ct_t[:, h * DH:(h + 1) * DH],
                            in0=x_t[:, h * DH:(h + 1) * DH],
                            scalar1=m_t[:, h:h + 1], scalar2=gam,
                            op0=AOP.subtract, op1=AOP.mult,
                        )

                # c2 = ct^2 (half tile on ACT), c3 = ct*c2 (half tile on DVE)
                hs = slice(half * (D // 2), (half + 1) * (D // 2))
                nc.scalar.activation(
                    out=c2_t[:, hs], in_=ct_t[:, hs],
                    func=mybir.ActivationFunctionType.Square,
                    scale=1.0,
                )
                nc.vector.tensor_tensor(
                    out=c3_t[:, hs], in0=ct_t[:, hs], in1=c2_t[:, hs],
                    op=AOP.mult,
                )

                for i, hp2 in enumerate(range(4 * half, 4 * half + 4)):
                    l_ps = l_pss[i]
                    if FOLD_C3:
                        for hh in range(2):
                            h = hp2 * 2 + hh
                            nc.tensor.matmul(
                                l_ps[:, hh * DH:(hh + 1) * DH],
                                lhsT=ident16, rhs=c3_t[:, h * DH:(h + 1) * DH],
                                start=False, stop=(hh == 1),
                                skip_group_check=True,
                            )
                    # drift (pair) PSUM -> SBUF fp16
                    _copy(
                        nc, DR_COPY_ENG[hp2],
                        dr_t[:, hp2 * 2 * DH:(hp2 + 1) * 2 * DH], l_ps
                    )
            if not FOLD_C3:
                # drift = dr + c3 in a full-tile pass (dr currently lin+bs)
                nc.vector.tensor_tensor(out=dr_t, in0=dr_t, in1=c3_t, op=AOP.add)

            # head-sum tree (order-independent pairwise sums)
            t8 = trp.tile([P, D // 2], FP16, tag="t8", name="t8")
            nc.vector.tensor_tensor(
                t8, in0=dr_t[:, 0:D // 2], in1=dr_t[:, D // 2:D], op=AOP.add
            )
            t4 = trp.tile([P, D // 4], FP16, tag="t4", name="t4")
            nc.gpsimd.tensor_tensor(
                t4, in0=t8[:, 0:D // 4], in1=t8[:, D // 4:D // 2], op=AOP.add
            )
            t2r = trp.tile([P, D // 8], FP16, tag="t2r", name="t2r")
            nc.vector.tensor_tensor(
                t2r, in0=t4[:, 0:D // 8], in1=t4[:, D // 8:D // 4], op=AOP.add
            )
            # mh2 = two side-by-side copies of -gp*sum_h(drift)
            mh2 = trp.tile([P, 2 * DH], FP16, tag="mh2", name="mh2")
            nc.vector.tensor_tensor(
                mh2[:, 0:DH], in0=t2r[:, 0:DH], in1=t2r[:, DH:2 * DH], op=AOP.add
            )
            nc.vector.tensor_scalar(
                out=mh2[:, 0:DH], in0=mh2[:, 0:DH], scalar1=-gp, scalar2=None,
                op0=AOP.mult,
            )
            nc.vector.tensor_copy(mh2[:, DH:2 * DH], mh2[:, 0:DH])

            # dd = drift + mh2 broadcast over pairs (single 2x pass)
            dd_t = c2_t  # reuse
            nc.vector.tensor_tensor(
                dd_t.rearrange("p (a b) -> p a b", a=H // 2),
                in0=dr_t.rearrange("p (a b) -> p a b", a=H // 2),
                in1=mh2.unsqueeze(1).broadcast_to([P, H // 2, 2 * DH]),
                op=AOP.add,
            )
            # out = x + dd
            o_t = op_.tile([P, D], FP16, tag="o", name="o_t")
            nc.vector.tensor_tensor(o_t, in0=x_t, in1=dd_t, op=AOP.add)
            if it % 2 == 0:
                nc.sync.dma_start(out=out_d[r0:r0 + P, :], in_=o_t)
            else:
                nc.scalar.dma_start(out=out_d[r0:r0 + P, :], in_=o_t)


_CACHE: dict = {}


def _build(cubic_scale: float, coupling: float) -> bass.Bass:
    key = (float(cubic_scale), float(coupling))
    if key in _CACHE:
        return _CACHE[key]
    nc = bacc.Bacc("TRN2", target_bir_lowering=False, debug=False)
    aps = {
        "state": nc.dram_tensor("state", [BS, D], FP16, kind="ExternalInput").ap(),
        "signal": nc.dram_tensor("signal", [BS, D], FP16, kind="ExternalInput").ap(),
        "U": nc.dram_tensor("U", [H, DH, R], F32, kind="ExternalInput").ap(),
        "V": nc.dram_tensor("V", [H, R, DH], F32, kind="ExternalInput").ap(),
        "diag": nc.dram_tensor("diag", [H, DH], F32, kind="ExternalInput").ap(),
        "out": nc.dram_tensor("out", [BS, D], FP16, kind="ExternalOutput").ap(),
    }
    with tile.TileContext(nc) as tc:
        _emit(tc, aps, float(cubic_scale), float(coupling))
    nc.compile()
    _CACHE[key] = nc
    return nc


def run(state, signal, U, V, diag, cubic_scale, coupling, trace=False):
    state = np.ascontiguousarray(np.asarray(state, dtype=np.float16))
    signal = np.ascontiguousarray(np.asarray(signal, dtype=np.float16))
    U = np.ascontiguousarray(np.asarray(U, dtype=np.float32))
    V = np.ascontiguousarray(np.asarray(V, dtype=np.float32))
    diag = np.ascontiguousarray(np.asarray(diag, dtype=np.float32))

    nc = _build(float(cubic_scale), float(coupling))
    in_maps = []
    for i in range(NCORES):
        sl = slice(i * BS, (i + 1) * BS)
        in_maps.append({
            "state": state[sl], "signal": signal[sl],
            "U": U, "V": V, "diag": diag,
        })
    res = run_bass_kernel_spmd(nc, in_maps, list(range(NCORES)), trace=trace)
    out = np.concatenate(
        [res.results[i]["out"] for i in range(NCORES)], axis=0
    ).astype(np.float32)
    return out, res


def kernel(state, signal, U, V, diag, cubic_scale, coupling) -> np.ndarray:
    out, _ = run(state, signal, U, V, diag, cubic_scale, coupling, trace=False)
    return out
